# revision 28
# baseline (speedup 1.0000x reference)
"""Trainium2 Bass kernel for TernaryLinear: y[b,m,n] = sum_k x[b,m,k] * w[k,n].

Shapes: x (4, 2048, 4096) fp32, w (4096, 4096) ternary fp32 -> y (4, 2048, 4096).

Strategy: flatten x to 8192 rows, row-shard across 8 NeuronCores (1024 rows
each), replicate w. All matmuls run in fp8e4 (e4m3, bias 8) with the
MatmulPerfMode.DoubleRow perf mode: each matmul instruction contracts K=256
(two 128-row k-subtiles in the middle AP dim). The PE moving port sustains
one 2-byte position per cycle, so a DoubleRow matmul retires 2 fp8 k-rows
per cycle -- 2x the bf16 MAC rate (measured 216 ns per K=256/N=512 matmul;
LDWEIGHTS fully hidden). The ternary weight {-1,0,1} is exact in fp8.

x is split into x_hi = e4m3(x) plus a residual x_lo = e4m3(x - x_hi); both
streams multiply the SAME w tiles and accumulate into the same PSUM group.
Correcting G of the 16 double-k-tiles costs G/16 extra matmuls and no extra
w traffic; rel err = 2.66e-2 * sqrt(1 - G/16) (G=10 -> 1.63e-2, measured).

Per core: x^T hi/lo resident in SBUF (hi on the scalar queue, lo on gpsimd;
the pipeline-gating first tile split into four 1 KB/partition quarters), w
streamed as per-j [128, 2, 512] fp8 tiles across the sync+scalar queues,
8 PSUM banks (one per 128-row m-tile), PSUM evicted as bf16 alternating
vector/scalar with output DMAs on gpsimd/scalar (host casts back to fp32).
No cross-core communication.
"""

import sys

for _p in ("/opt/trn_rl_repo", "/opt/pypackages"):
    if _p not in sys.path:
        sys.path.append(_p)

import ml_dtypes
import numpy as np

import concourse.bass as bass
import concourse.bacc as bacc
import concourse.mybir as mybir
import concourse.tile as tile
from concourse.bass_utils import run_bass_kernel_spmd

P = 128
NCORES = 8
B, M, K, N = 4, 2048, 4096, 4096
R = B * M            # 8192 rows total
MR = R // NCORES     # 1024 rows per core
DKT = K // (2 * P)   # 16 double-k-tiles (256 contraction each)
MT = MR // P         # 8 m-tiles per core
NCH = 512            # moving free dim per matmul (one PSUM bank of fp32)
NCHUNKS = N // NCH   # 8
G = 10               # double-k-tiles that get the x_lo residual correction
XG = 2               # double-k-tiles per x DMA / resident tile
NXT = DKT // XG      # 8 hi x tiles
NXL = G // XG        # 5 lo x tiles (G must be a multiple of XG)
WLG = 4              # double-k-tiles per w tile in the last n-chunk
F32 = mybir.dt.float32
BF16 = mybir.dt.bfloat16
F8 = mybir.dt.float8e4
E4M3 = ml_dtypes.float8_e4m3
DR = mybir.MatmulPerfMode.DoubleRow

_PROGRAM = None


def _build_program():
    nc = bacc.Bacc(
        "TRN2",
        target_bir_lowering=False,
        debug=False,
        num_devices=NCORES,
    )
    xt = nc.dram_tensor("xt", [P, NXT, XG, 2, MT, P], F8, kind="ExternalInput").ap()
    xl = nc.dram_tensor("xl", [P, NXL, XG, 2, MT, P], F8, kind="ExternalInput").ap()
    w = nc.dram_tensor("w", [NCHUNKS, DKT, P, 2, NCH], F8, kind="ExternalInput").ap()
    y = nc.dram_tensor("y", [MT, P, N], BF16, kind="ExternalOutput").ap()

    with tile.TileContext(nc) as tc:
        with (
            tc.tile_pool(name="xres", bufs=1) as xpool,
            tc.tile_pool(name="wstream", bufs=12) as wpool,
            tc.tile_pool(name="outstage", bufs=8) as opool,
            tc.tile_pool(name="acc", bufs=8, space="PSUM") as ppool,
        ):
            # x^T resident: hi and lo tiles, [128 kp, XG, 2, MT, 128 m] each,
            # loaded on the scalar queue while the sync queue streams chunk 0's
            # w tiles, so the PE starts after one x tile + one w tile.
            xtiles = [None] * NXT
            xltiles = [None] * NXL

            def evict(nch, mt, ps, last=False):
                ot = opool.tile([P, NCH], BF16, tag="o", name=f"o{nch}_{mt}")
                if last:
                    # tail-critical: DVE copy with the DMA pre-issued on the
                    # (idle) sync queue so the issue overhead overlaps the copy
                    nc.vector.tensor_copy(ot[:], ps[:])
                    nc.sync.dma_start(out=y[mt, :, bass.ts(nch, NCH)], in_=ot[:])
                    return
                if mt % 2 == 0:
                    nc.vector.tensor_copy(ot[:], ps[:])
                    dma_eng = nc.gpsimd
                else:
                    nc.scalar.copy(ot[:], ps[:])
                    dma_eng = nc.scalar
                dma_eng.dma_start(out=y[mt, :, bass.ts(nch, NCH)], in_=ot[:])

            def load_x(xg):
                if xg == 0:
                    # the first tile gates the whole pipeline: load it as four
                    # independent quarter tiles (per k-subtile x per m-half)
                    # so the first matmul only waits for a 1 KB/partition DMA
                    quarters = []
                    for jj in range(XG):
                        row = []
                        for h in range(2):
                            qt = xpool.tile(
                                [P, 2, MT // 2, P], F8,
                                tag=f"x0_{jj}{h}", name=f"x0_{jj}{h}",
                            )
                            nc.scalar.dma_start(
                                out=qt[:],
                                in_=xt[:, 0, jj, :, bass.ts(h, MT // 2)],
                            )
                            row.append(qt)
                        quarters.append(row)
                        if jj == 0:
                            # xl0 is deadline-critical (first lo matmul is 8
                            # matmuls in); on gpsimd's slow SWDGE path it
                            # arrives ~2us late and stalls the PE. Slot it on
                            # scalar right between the j=0 and j=1 quarters.
                            xlt = xpool.tile(
                                [P, XG, 2, MT, P], F8, tag="xl0", name="xl0"
                            )
                            nc.scalar.dma_start(out=xlt[:], in_=xl[:, 0])
                            xltiles[0] = xlt
                    xtiles[0] = quarters
                else:
                    xtile = xpool.tile(
                        [P, XG, 2, MT, P], F8, tag=f"x{xg}", name=f"x{xg}"
                    )
                    # the chunk-0 x preload alone saturates the scalar queue
                    # (delivery ~= the PE's consumption rate, zero slack), so
                    # alternate pairs onto the sync queue, which has ~50%
                    # headroom under the w stream
                    (nc.scalar if xg % 2 == 0 else nc.sync).dma_start(
                        out=xtile[:], in_=xt[:, xg]
                    )
                    xtiles[xg] = xtile
                if 0 < xg < NXL:
                    xltile = xpool.tile(
                        [P, XG, 2, MT, P], F8, tag=f"xl{xg}", name=f"xl{xg}"
                    )
                    # later lo tiles have large deadline slack; ride the
                    # otherwise-idle gpsimd queue
                    nc.gpsimd.dma_start(out=xltile[:], in_=xl[:, xg])
                    xltiles[xg] = xltile

            def hi_lhsT(xg, jj, mt):
                if xg == 0:
                    return xtiles[0][jj][mt // (MT // 2)][:, :, mt % (MT // 2), :]
                return xtiles[xg][:, jj, :, mt, :]

            def mms_hi(psums, wt_j, j, mt_range):
                xg, jj = divmod(j, XG)
                for mt in mt_range:
                    nc.tensor.matmul(
                        out=psums[mt][:],
                        lhsT=hi_lhsT(xg, jj, mt),
                        rhs=wt_j,
                        start=(j == 0),
                        stop=(j == DKT - 1 and G < DKT),
                        perf_mode=DR,
                    )

            def mms_lo(psums, wt_j, j, mt_range):
                xg, jj = divmod(j, XG)
                for mt in mt_range:
                    nc.tensor.matmul(
                        out=psums[mt][:],
                        lhsT=xltiles[xg][:, jj, :, mt, :],
                        rhs=wt_j,
                        start=False,
                        stop=(j == DKT - 1),
                        perf_mode=DR,
                    )

            def mms(psums, wt_j, j, mt_range):
                mms_hi(psums, wt_j, j, mt_range)
                if j < G:
                    mms_lo(psums, wt_j, j, mt_range)

            for nch in range(NCHUNKS - 1):
                psums = [
                    ppool.tile([P, NCH], F32, tag="acc", name=f"ps{nch}_{mt}")
                    for mt in range(MT)
                ]
                for j in range(DKT):
                    wt = wpool.tile([P, 2, NCH], F8, tag="w", name=f"w{nch}_{j}")
                    # chunk 0: scalar queue is busy with the x preload, keep w
                    # on sync; afterwards alternate the two queues.
                    weng = nc.sync if (nch == 0 or j % 2 == 0) else nc.scalar
                    weng.dma_start(out=wt[:], in_=w[nch, j])
                    if nch == 0 and j % XG == 0:
                        load_x(j // XG)
                    mms(psums, wt[:], j, range(MT))
                for mt in range(MT):
                    evict(nch, mt, psums[mt])

            # Last n-chunk: mt-outer / k-inner so each m-tile's accumulation
            # finishes early and its eviction + output DMA overlap the
            # remaining matmul stream; only the last m-tile drains after the
            # final matmul. Its w tiles are pinned (all 16 j live at once).
            nch = NCHUNKS - 1
            wlast = []
            for wg in range(DKT // WLG):
                wt = wpool.tile(
                    [P, WLG, 2, NCH], F8, tag=f"wl{wg}", name=f"wl{wg}", bufs=1
                )
                for i in range(WLG):
                    (nc.sync if (wg * WLG + i) % 2 == 0 else nc.scalar).dma_start(
                        out=wt[:, i], in_=w[nch, wg * WLG + i]
                    )
                wlast.append(wt)
            for mt in range(MT):
                ps = ppool.tile([P, NCH], F32, tag="acc", name=f"psL_{mt}")
                for j in range(DKT):
                    mms([ps] * MT, wlast[j // WLG][:, j % WLG], j, [mt])
                evict(nch, mt, ps, last=(mt == MT - 1))
    nc.compile()
    return nc


def _get_program():
    global _PROGRAM
    if _PROGRAM is None:
        _PROGRAM = _build_program()
    return _PROGRAM


def _prepare_in_maps(x: np.ndarray, w: np.ndarray):
    x = np.ascontiguousarray(x, dtype=np.float32).reshape(R, K)
    w = np.ascontiguousarray(w, dtype=np.float32)
    x_hi = x.astype(E4M3)
    x_lo = (x - x_hi.astype(np.float32)).astype(E4M3)

    # rows -> [core, mt, mp, xg, jj, sub, kp] -> [core, kp, xg, jj, sub, mt, mp]
    def pack_x(a, ntiles):
        ar = a.reshape(NCORES, MT, P, NXT, XG, 2, P)
        return np.ascontiguousarray(ar.transpose(0, 6, 3, 4, 5, 1, 2)[:, :, :ntiles])

    xt_all = pack_x(x_hi, NXT)
    xl_all = pack_x(x_lo, NXL)
    # w [j, sub, kp, nch, nn] -> [nch, j, kp, sub, nn]
    wr = np.ascontiguousarray(
        w.reshape(DKT, 2, P, NCHUNKS, NCH).transpose(3, 0, 2, 1, 4).astype(E4M3)
    )
    return [
        {"xt": xt_all[c], "xl": xl_all[c], "w": wr}
        for c in range(NCORES)
    ]


def _gather_output(results):
    y = np.stack([np.asarray(r["y"]) for r in results])  # [core, MT, P, N] bf16
    return y.astype(np.float32).reshape(B, M, N)


def run(x: np.ndarray, w: np.ndarray, trace: bool = False):
    """Returns (y, BassKernelResults)."""
    nc = _get_program()
    in_maps = _prepare_in_maps(x, w)
    res = run_bass_kernel_spmd(
        nc, in_maps, core_ids=list(range(NCORES)), trace=trace
    )
    return _gather_output(res.results), res


def kernel(x: np.ndarray, w: np.ndarray) -> np.ndarray:
    y, _ = run(x, w, trace=False)
    return y
